# revision 14
# baseline (speedup 1.0000x reference)
"""Trainium2 Bass kernel for a pre-LN transformer decoder block.

Reference computation (per batch b):
    h  = LN1(x);  qkv = h@W_qkv + b;  causal attention (16 heads, d=64)
    x  = x + attn@W_o + b_o
    h  = LN2(x);  x = x + gelu(h@W_fc + b_fc)@W_proj + b_proj

Sharding: 8 cores = 2 batches x 4 query chunks of 512 tokens.  Each core
receives its batch's context front-padded with zeros to 2048 tokens
(queries always sit at local tokens 1536:2048), computes K/V for the whole
local context on-device, and emits the final 512 output rows.  No
cross-core communication.

Layout: everything feature-major (features on SBUF partitions), transposed
on the host.  Matmuls run as float32r (fp32 storage rounded to FP22 on
write, fp32 PSUM accumulate) — full PE rate at N>=256.  Scores are
computed transposed S^T[k, q] so softmax normalization folds into
per-partition ops + one tiny broadcast matmul.  Padded keys yield
exactly-zero scores; their exp(0)=1 contribution is removed by
subtracting the per-core pad count from the softmax denominator.
"""

import sys

if "/opt/trn_rl_repo" not in sys.path:
    sys.path.insert(0, "/opt/trn_rl_repo")

import numpy as np

N_EMBD = 1024
N_HEAD = 16
HEAD_DIM = 64
B, T = 2, 2048
NC = 8  # cores
CHUNK = 512  # query tokens per core
PT = 2048  # padded local context
ET = N_EMBD // 128  # 8 embedding tiles
FT = 4 * N_EMBD // 128  # 32 mlp tiles
KT = PT // 128  # 16 key tiles
TC = PT // 512  # 4 token chunks of 512
VW = 65 * N_HEAD  # 1040: v with interleaved ones columns
EPS = 1e-5


def build(stage=3, dbg=False, reps=1):
    import concourse.mybir as mybir
    import concourse.tile as tile
    from concourse import bacc

    f32 = mybir.dt.float32
    f32r = mybir.dt.float32r

    nc = bacc.Bacc("TRN2", target_bir_lowering=False, debug=False, num_devices=NC)

    g = {}
    g["xT"] = nc.declare_dram_parameter("xT", [ET, 128, PT], f32, isOutput=False)
    g["wq"] = nc.declare_dram_parameter("wq", [ET, 128, N_EMBD], f32, isOutput=False)
    g["wk"] = nc.declare_dram_parameter("wk", [ET, 128, N_EMBD], f32, isOutput=False)
    g["wv"] = nc.declare_dram_parameter("wv", [ET, 128, VW], f32, isOutput=False)
    g["wo"] = nc.declare_dram_parameter("wo", [ET, 128, N_EMBD], f32, isOutput=False)
    g["wfc"] = nc.declare_dram_parameter("wfc", [ET, 128, 4 * N_EMBD], f32, isOutput=False)
    g["wpr"] = nc.declare_dram_parameter("wpr", [FT, 128, N_EMBD], f32, isOutput=False)
    for nm, n in (("bq", ET), ("bk", ET), ("bo", ET), ("bfc", FT), ("bpr", ET),
                  ("g1", ET), ("b1", ET), ("g2", ET), ("b2", ET)):
        g[nm] = nc.declare_dram_parameter(nm, [128, n], f32, isOutput=False)
    g["causal"] = nc.declare_dram_parameter("causal", [4, 128, CHUNK], f32, isOutput=False)
    g["sel"] = nc.declare_dram_parameter("sel", [2, 128], f32, isOutput=False)
    g["npad"] = nc.declare_dram_parameter("npad", [1, 1], f32, isOutput=False)

    g["yT"] = nc.declare_dram_parameter("yT", [ET, 128, CHUNK], f32, isOutput=True)
    dbg_out = {}
    if dbg:
        dbg_out["lnxT"] = nc.declare_dram_parameter("d_lnxT", [ET, 128, PT], f32, isOutput=True)
        dbg_out["kT"] = nc.declare_dram_parameter("d_kT", [ET, 128, PT], f32, isOutput=True)
        dbg_out["qT"] = nc.declare_dram_parameter("d_qT", [ET, 128, CHUNK], f32, isOutput=True)
        dbg_out["v"] = nc.declare_dram_parameter("d_v", [KT, 128, VW], f32, isOutput=True)
        if stage >= 2:
            dbg_out["attnT"] = nc.declare_dram_parameter("d_attnT", [ET, 128, CHUNK], f32, isOutput=True)
            dbg_out["x2T"] = nc.declare_dram_parameter("d_x2T", [ET, 128, CHUNK], f32, isOutput=True)

    g["kts"] = nc.dram_tensor("kts", [ET, 128, PT], f32r)  # K^T spill
    g["attns"] = nc.dram_tensor("attns", [ET, 128, CHUNK], f32r)  # attn^T spill

    with tile.TileContext(nc) as tc:
        with (
            tc.tile_pool(name="const", bufs=1) as cpool,
            tc.tile_pool(name="resident", bufs=1) as rpool,
        ):
            ones_tmp = cpool.tile([128, 16], f32, tag="ones_tmp", name="ones_tmp")
            nc.vector.memset(ones_tmp[:, :], 1.0)
            g["ones16"] = ones_tmp
            g["ones_p"] = cpool.tile([128, 1], f32r, tag="ones_p", name="ones_p")
            g["ones_f"] = cpool.tile([1, 128], f32r, tag="ones_f", name="ones_f")
            nc.vector.tensor_copy(g["ones_p"][:, :], ones_tmp[:, 0:1])
            nc.vector.tensor_copy(g["ones_f"][:, :], ones_tmp[0:1, 0:1].to_broadcast([1, 128]))
            g["eps_sb"] = cpool.tile([128, 1], f32, tag="eps_sb", name="eps_sb")
            nc.vector.memset(g["eps_sb"][:, :], EPS)
            for nm, shp, dt_ in (("sel_sb", [2, 128], f32r), ("np_sb", [1, 1], f32),
                                 ("cau_sb", [128, 4, CHUNK], f32),
                                 ("bq_sb", [128, ET], f32), ("bk_sb", [128, ET], f32),
                                 ("bo_sb", [128, ET], f32), ("bfc_sb", [128, FT], f32),
                                 ("bpr_sb", [128, ET], f32), ("g1_sb", [128, ET], f32),
                                 ("b1_sb", [128, ET], f32), ("g2_sb", [128, ET], f32),
                                 ("b2_sb", [128, ET], f32)):
                g[nm] = cpool.tile(shp, dt_, tag=nm, name=nm)
            f32r_tiles = {"sel_sb"}
            for s_, t_ in (("sel_sb", "sel"), ("np_sb", "npad"), ("cau_sb", "causal"),
                           ("bq_sb", "bq"), ("bk_sb", "bk"), ("bo_sb", "bo"),
                           ("bfc_sb", "bfc"), ("bpr_sb", "bpr"), ("g1_sb", "g1"),
                           ("b1_sb", "b1"), ("g2_sb", "g2"), ("b2_sb", "b2")):
                src = g[t_]
                dst = g[s_]
                src_ap = src[tuple(slice(None) for _ in src.shape)]
                if s_ == "cau_sb":
                    src_ap = src_ap.transpose([1, 0, 2])
                if s_ in f32r_tiles:
                    src_ap = src_ap.bitcast(f32r)
                nc.sync.dma_start(out=dst[tuple(slice(None) for _ in dst.shape)],
                                  in_=src_ap)

            with nc.allow_low_precision(reason="fp32r (fp22) matmul operands by design"):
                for _rep in range(reps):
                    _body(nc, tc, g, stage, dbg, dbg_out)

    nc.compile()
    return nc, dbg_out


def _body(nc, tc, g, stage, dbg, dbg_out):
    import concourse.mybir as mybir

    f32 = mybir.dt.float32
    f32r = mybir.dt.float32r
    AF = mybir.ActivationFunctionType
    OP = mybir.AluOpType

    def b_(ap):  # read an f32r AP on DVE/ACT
        return ap.bitcast(f32)

    from contextlib import ExitStack

    xT, kts, yT = g["xT"], g["kts"], g["yT"]
    ones_p, ones_f = g["ones_p"], g["ones_f"]
    qr_stack = ExitStack()
    qrpool = qr_stack.enter_context(tc.tile_pool(name="qkv_res", bufs=1))
    v_t = [qrpool.tile([128, VW], f32r, tag=f"v{i}", name=f"v{i}") for i in range(KT)]
    qT_t = [qrpool.tile([128, CHUNK], f32r, tag=f"qT{i}", name=f"qT{i}") for i in range(ET)]

    def kgetw(w, csl):
        return w[:, :, csl].transpose([1, 0, 2]).bitcast(f32r)

    def layernorm_feature_major(pool, pspool, src_tiles_or_dram, width, out_tiles,
                                g_sb, b_sb, pfx, dram_mode):
        """LN over features (partitions); writes out_tiles[et][:, sl] (f32r)."""
        n_ch = width // 512
        for tci in range(n_ch):
            sl = slice(512 * tci, 512 * (tci + 1))

            def get_src(et):
                if dram_mode:
                    xt = pool.tile([128, 512], f32r, tag=pfx + "xt", name=pfx + "xt")
                    nc.sync.dma_start(out=xt[:, :], in_=src_tiles_or_dram[et, :, sl].bitcast(f32r))
                    return xt[:, :]
                return src_tiles_or_dram[et][:, sl]

            ps_s = pspool.tile([1, 512], f32, tag=pfx + "s", name=pfx + "s")
            ps_q = pspool.tile([1, 512], f32, tag=pfx + "q", name=pfx + "q")
            for et in range(ET):
                xt = get_src(et)
                sq = pool.tile([128, 512], f32r, tag=pfx + "sq", name=pfx + "sq")
                nc.vector.tensor_mul(sq[:, :], b_(xt), b_(xt))
                nc.tensor.matmul(ps_s[:, :], ones_p[:, :], xt,
                                 start=(et == 0), stop=(et == ET - 1))
                nc.tensor.matmul(ps_q[:, :], ones_p[:, :], sq[:, :],
                                 start=(et == 0), stop=(et == ET - 1))
            mu = pool.tile([1, 512], f32r, tag=pfx + "mu", name=pfx + "mu")
            nc.scalar.activation(mu[:, :], ps_s[:, :], AF.Copy, scale=1.0 / N_EMBD)
            musq = pool.tile([1, 512], f32, tag=pfx + "musq", name=pfx + "musq")
            nc.vector.tensor_mul(musq[:, :], b_(mu[:, :]), b_(mu[:, :]))
            var = pool.tile([1, 512], f32, tag=pfx + "var", name=pfx + "var")
            nc.scalar.activation(var[:, :], ps_q[:, :], AF.Copy, scale=1.0 / N_EMBD)
            nc.vector.tensor_sub(var[:, :], var[:, :], musq[:, :])
            sd = pool.tile([1, 512], f32, tag=pfx + "sd", name=pfx + "sd")
            nc.scalar.activation(sd[:, :], var[:, :], AF.Sqrt, bias=g["eps_sb"][0:1, 0:1])
            rstd = pool.tile([1, 512], f32r, tag=pfx + "rstd", name=pfx + "rstd")
            nc.vector.reciprocal(rstd[:, :], sd[:, :])
            ps_mb = pspool.tile([128, 512], f32, tag=pfx + "mb", name=pfx + "mb")
            ps_rb = pspool.tile([128, 512], f32, tag=pfx + "rb", name=pfx + "rb")
            nc.tensor.matmul(ps_mb[:, :], ones_f[:, :], mu[:, :], start=True, stop=True)
            nc.tensor.matmul(ps_rb[:, :], ones_f[:, :], rstd[:, :], start=True, stop=True)
            mu_b = pool.tile([128, 512], f32, tag=pfx + "mu_b", name=pfx + "mu_b")
            rs_b = pool.tile([128, 512], f32, tag=pfx + "rs_b", name=pfx + "rs_b")
            nc.vector.tensor_copy(mu_b[:, :], ps_mb[:, :])
            nc.vector.tensor_copy(rs_b[:, :], ps_rb[:, :])
            for et in range(ET):
                xt = get_src(et)
                t1 = pool.tile([128, 512], f32, tag=pfx + "t1", name=pfx + "t1")
                nc.vector.tensor_sub(t1[:, :], b_(xt), mu_b[:, :])
                nc.vector.tensor_mul(t1[:, :], t1[:, :], rs_b[:, :])
                nc.vector.tensor_scalar(out_tiles[et][:, sl], t1[:, :],
                                        g_sb[:, et:et + 1], b_sb[:, et:et + 1],
                                        OP.mult, OP.add)

    # ---------------- Phase A: LN1 + QKV ----------------
    with tc.tile_pool(name="lnx_pool", bufs=1) as lpool:
        lnxT = [lpool.tile([128, PT], f32r, tag=f"lnxT{i}", name=f"lnxT{i}") for i in range(ET)]
        with (
            tc.tile_pool(name="pa_sb", bufs=2) as pa,
            tc.tile_pool(name="pa_ps", bufs=2, space="PSUM") as pps,
        ):
            layernorm_feature_major(pa, pps, xT, PT, lnxT,
                                    g["g1_sb"], g["b1_sb"], "a", dram_mode=True)

        with (
            tc.tile_pool(name="qkv_w", bufs=2) as wpool,
            tc.tile_pool(name="qkv_sb", bufs=4) as spool,
            tc.tile_pool(name="qkv_ps", bufs=4, space="PSUM") as qps,
        ):
            for panel in range(4):
                csl = slice(256 * panel, 256 * (panel + 1))
                wk_p = wpool.tile([128, ET, 256], f32r, tag="wpan", name="wk_p")
                nc.sync.dma_start(out=wk_p[:, :, :], in_=kgetw(g["wk"], csl))
                for fto in range(2):
                    ft = 2 * panel + fto
                    for tci in range(TC):
                        ps = qps.tile([128, 512], f32, tag="mmps", name="kq_ps")
                        for et in range(ET):
                            nc.tensor.matmul(ps[:, :], wk_p[:, et, 128 * fto:128 * (fto + 1)],
                                             lnxT[et][:, 512 * tci:512 * (tci + 1)],
                                             start=(et == 0), stop=(et == ET - 1))
                        kt_sb = spool.tile([128, 512], f32r, tag="kt_sb", name="kt_sb")
                        nc.vector.tensor_scalar(kt_sb[:, :], ps[:, :],
                                                g["bk_sb"][:, ft:ft + 1], None, OP.add)
                        nc.sync.dma_start(out=kts[ft, :, 512 * tci:512 * (tci + 1)], in_=kt_sb[:, :])
                        if dbg:
                            nc.sync.dma_start(out=dbg_out["kT"][ft, :, 512 * tci:512 * (tci + 1)],
                                              in_=b_(kt_sb[:, :]))
                wq_p = wpool.tile([128, ET, 256], f32r, tag="wpan", name="wq_p")
                nc.sync.dma_start(out=wq_p[:, :, :], in_=kgetw(g["wq"], csl))
                for fto in range(2):
                    ft = 2 * panel + fto
                    ps = qps.tile([128, 512], f32, tag="mmps", name="q_ps")
                    for et in range(ET):
                        nc.tensor.matmul(ps[:, :], wq_p[:, et, 128 * fto:128 * (fto + 1)],
                                         lnxT[et][:, 1536:2048],
                                         start=(et == 0), stop=(et == ET - 1))
                    nc.vector.tensor_scalar(qT_t[ft][:, :], ps[:, :],
                                            g["bq_sb"][:, ft:ft + 1], None, OP.add)
                    if dbg:
                        nc.sync.dma_start(out=dbg_out["qT"][ft, :, :], in_=b_(qT_t[ft][:, :]))
            for c0, cw in ((0, 256), (256, 256), (512, 256), (768, 256), (1024, 16)):
                wv_p = wpool.tile([128, ET, 256], f32r, tag="wpan", name="wv_p")
                nc.sync.dma_start(out=wv_p[:, :, :cw], in_=kgetw(g["wv"], slice(c0, c0 + cw)))
                for tt in range(KT):
                    ps = qps.tile([128, 256], f32, tag="vps", name="v_ps")
                    for et in range(ET):
                        nc.tensor.matmul(ps[:, :cw], lnxT[et][:, 128 * tt:128 * (tt + 1)],
                                         wv_p[:, et, :cw],
                                         start=(et == 0), stop=(et == ET - 1))
                    nc.vector.tensor_copy(v_t[tt][:, c0:c0 + cw], ps[:, :cw])
            for tt in range(KT):  # ones columns feeding the softmax denominator
                nc.vector.tensor_copy(v_t[tt][:, 64:VW:65], g["ones16"][:, :])
                if dbg:
                    nc.sync.dma_start(out=dbg_out["v"][tt, :, :], in_=b_(v_t[tt][:, :]))
            if dbg:
                for et in range(ET):
                    nc.sync.dma_start(out=dbg_out["lnxT"][et, :, :], in_=b_(lnxT[et][:, :]))

    if stage < 2:
        for et in range(ET):
            nc.sync.dma_start(out=yT[et, :, :], in_=b_(qT_t[et][:, :]))
        qr_stack.close()
        return

    # ---------------- Phase B: attention ----------------
    attns = g["attns"]
    if True:
        with (
            tc.tile_pool(name="at_sb", bufs=2) as apool,
            tc.tile_pool(name="at_exp", bufs=3) as epool,
            tc.tile_pool(name="at_ps", bufs=2, space="PSUM") as sps,
            tc.tile_pool(name="at_av", bufs=2, space="PSUM") as avps,
            tc.tile_pool(name="at_bc", bufs=1, space="PSUM") as bcps,
        ):
            for hp in range(N_HEAD // 2):
                kp = apool.tile([128, PT], f32r, tag="kp", name="kp")
                nc.sync.dma_start(out=kp[:, :], in_=kts[hp, :, :])
                ps_a = [None, None]
                recip_t = [None, None]
                for h2 in range(2):
                    po = 64 * h2
                    h = 2 * hp + h2
                    ps_a[h2] = avps.tile([65, 512], f32, tag="av", name="av_ps")
                    for kt2 in range(KT // 2):
                        ps_s = sps.tile([128, 1024], f32, tag="scores", name="s_ps")
                        ex = epool.tile([128, 1024], f32r, tag="expS", name="expS")
                        for j in range(2):
                            kt = 2 * kt2 + j
                            nc.tensor.matmul(ps_s[:, 512 * j:512 * (j + 1)],
                                             kp[po:po + 64, 128 * kt:128 * (kt + 1)],
                                             qT_t[hp][po:po + 64, :],
                                             start=True, stop=True)
                            if kt >= 12:
                                nc.vector.tensor_add(ps_s[:, 512 * j:512 * (j + 1)],
                                                     ps_s[:, 512 * j:512 * (j + 1)],
                                                     g["cau_sb"][:, kt - 12, :])
                        nc.scalar.activation(ex[:, :], ps_s[:, :], AF.Exp)
                        for j in range(2):
                            kt = 2 * kt2 + j
                            nc.tensor.matmul(ps_a[h2][:, :],
                                             v_t[kt][:, 65 * h:65 * h + 65],
                                             ex[:, 512 * j:512 * (j + 1)],
                                             start=(kt == 0), stop=(kt == KT - 1),
                                             skip_group_check=True)
                    den = apool.tile([1, 512], f32, tag="den", name="den")
                    nc.vector.tensor_scalar(den[:, :], ps_a[h2][64:65, :],
                                            g["np_sb"][0:1, 0:1], None, OP.subtract)
                    recip_t[h2] = apool.tile([1, 512], f32r, tag=f"recip{h2}", name=f"recip{h2}")
                    nc.vector.reciprocal(recip_t[h2][:, :], den[:, :])
                rb_sb = apool.tile([128, 512], f32, tag="rb_sb", name="rb_sb")
                for h2 in range(2):
                    ps_b = bcps.tile([64, 512], f32, tag="bc", name="bc_ps")
                    nc.tensor.matmul(ps_b[:, :], ones_f[0:1, 0:64],
                                     recip_t[h2][:, :], start=True, stop=True)
                    nc.vector.tensor_copy(rb_sb[64 * h2:64 * h2 + 64, :], ps_b[:, :])
                at_sb = apool.tile([128, CHUNK], f32r, tag="at_sb_t", name="at_sb_t")
                for h2 in range(2):
                    po = 64 * h2
                    nc.vector.tensor_mul(at_sb[po:po + 64, :], ps_a[h2][0:64, :],
                                         rb_sb[po:po + 64, :])
                nc.sync.dma_start(out=attns[hp, :, :], in_=at_sb[:, :])
                if dbg:
                    nc.sync.dma_start(out=dbg_out["attnT"][hp, :, :], in_=b_(at_sb[:, :]))
        qr_stack.close()  # v/q dead from here; free 82KB for phase C

        # ---------------- Phase C: W_o + residual + LN2 + MLP ----------------
        with (
            tc.tile_pool(name="pc_sb", bufs=2) as pc,
            tc.tile_pool(name="pc_res", bufs=1) as pcr,
            tc.tile_pool(name="pc_w", bufs=2) as pw,
            tc.tile_pool(name="pc_ps", bufs=4, space="PSUM") as cps,
            tc.tile_pool(name="pc_ps2", bufs=1, space="PSUM") as cps2,
        ):
            x2T = [pcr.tile([128, CHUNK], f32r, tag=f"x2T{i}", name=f"x2T{i}") for i in range(ET)]
            ln2T = [pcr.tile([128, CHUNK], f32r, tag=f"ln2T{i}", name=f"ln2T{i}") for i in range(ET)]
            hT = [pcr.tile([128, CHUNK], f32r, tag=f"hT{i}", name=f"hT{i}") for i in range(FT)]
            attnT = [pcr.tile([128, CHUNK], f32r, tag=f"atT{i}", name=f"atT{i}") for i in range(ET)]
            for et in range(ET):
                nc.sync.dma_start(out=attnT[et][:, :], in_=attns[et, :, :])
            for panel in range(4):
                csl = slice(256 * panel, 256 * (panel + 1))
                wo_p = pw.tile([128, ET, 256], f32r, tag="cwpan", name="wo_p")
                nc.sync.dma_start(out=wo_p[:, :, :], in_=kgetw(g["wo"], csl))
                for fto in range(2):
                    ot = 2 * panel + fto
                    ps = cps.tile([128, 512], f32, tag="cmmps", name="wo_ps")
                    for et in range(ET):
                        nc.tensor.matmul(ps[:, :], wo_p[:, et, 128 * fto:128 * (fto + 1)],
                                         attnT[et][:, :],
                                         start=(et == 0), stop=(et == ET - 1))
                    xq = pc.tile([128, 512], f32, tag="xq", name="xq")
                    nc.sync.dma_start(out=xq[:, :], in_=xT[ot, :, 1536:2048])
                    nc.vector.scalar_tensor_tensor(x2T[ot][:, :], ps[:, :],
                                                   g["bo_sb"][:, ot:ot + 1], xq[:, :],
                                                   OP.add, OP.add)
            if dbg:
                for et in range(ET):
                    nc.sync.dma_start(out=dbg_out["x2T"][et, :, :], in_=b_(x2T[et][:, :]))

            layernorm_feature_major(pc, cps2, x2T, CHUNK, ln2T,
                                    g["g2_sb"], g["b2_sb"], "c", dram_mode=False)

            for panel in range(16):
                csl = slice(256 * panel, 256 * (panel + 1))
                wf_p = pw.tile([128, ET, 256], f32r, tag="cwpan", name="wf_p")
                nc.sync.dma_start(out=wf_p[:, :, :], in_=kgetw(g["wfc"], csl))
                for fto in range(2):
                    ft = 2 * panel + fto
                    ps = cps.tile([128, 512], f32, tag="cmmps", name="fc_ps")
                    for et in range(ET):
                        nc.tensor.matmul(ps[:, :], wf_p[:, et, 128 * fto:128 * (fto + 1)],
                                         ln2T[et][:, :],
                                         start=(et == 0), stop=(et == ET - 1))
                    nc.scalar.activation(hT[ft][:, :], ps[:, :], AF.Gelu,
                                         bias=g["bfc_sb"][:, ft:ft + 1])

            for panel in range(4):  # proj output features, 256 (= 2 e-tiles) each
                csl = slice(256 * panel, 256 * (panel + 1))
                pso = [cps.tile([128, 512], f32, tag="cmmps", name=f"pr_ps{i}") for i in range(2)]
                for ftg in range(4):
                    wp_p = pw.tile([128, ET, 256], f32r, tag="cwpan", name="wp_p")
                    nc.sync.dma_start(
                        out=wp_p[:, :, :],
                        in_=g["wpr"][8 * ftg:8 * (ftg + 1), :, csl].transpose([1, 0, 2]).bitcast(f32r))
                    for fl in range(ET):
                        ft = 8 * ftg + fl
                        for fto in range(2):
                            nc.tensor.matmul(pso[fto][:, :],
                                             wp_p[:, fl, 128 * fto:128 * (fto + 1)],
                                             hT[ft][:, :],
                                             start=(ft == 0), stop=(ft == FT - 1),
                                             skip_group_check=True)
                for fto in range(2):
                    et_o = 2 * panel + fto  # output e-tile index
                    out_sb = pc.tile([128, 512], f32, tag="out_sb", name="out_sb")
                    nc.vector.scalar_tensor_tensor(out_sb[:, :], pso[fto][:, :],
                                                   g["bpr_sb"][:, et_o:et_o + 1],
                                                   b_(x2T[et_o][:, :]), OP.add, OP.add)
                    nc.sync.dma_start(out=yT[et_o, :, :], in_=out_sb[:, :])


def _prep_inputs(x, ln1_g, ln1_b, ln2_g, ln2_b, W_qkv, b_qkv, W_o, b_o, W_fc, b_fc, W_proj, b_proj):
    """Host-side shard prep. Returns list of 8 in_maps."""
    f = np.float32
    x = np.asarray(x, f)
    W_qkv = np.asarray(W_qkv, f)
    b_qkv = np.asarray(b_qkv, f)
    scale = f(1.0) / f(np.sqrt(HEAD_DIM))
    Wq = W_qkv[:, :N_EMBD] * scale
    Wk = W_qkv[:, N_EMBD:2 * N_EMBD]
    Wv = W_qkv[:, 2 * N_EMBD:]
    bqv = b_qkv[:N_EMBD] * scale
    bkv = b_qkv[N_EMBD:2 * N_EMBD]
    bvv = b_qkv[2 * N_EMBD:]
    Wv2 = np.zeros((N_EMBD, VW), f)
    for h in range(N_HEAD):
        Wv2[:, 65 * h:65 * h + 64] = Wv[:, 64 * h:64 * h + 64]
    bo2 = np.asarray(b_o, f) + bvv @ np.asarray(W_o, f)

    def ptile(vec, n):
        return np.ascontiguousarray(np.asarray(vec, f).reshape(n, 128).T)

    def wtile(w, n):
        return np.ascontiguousarray(np.asarray(w, f).reshape(n, 128, -1))

    causal = np.where(np.arange(CHUNK)[:, None] > np.arange(CHUNK)[None, :],
                      f(-1e9), f(0.0)).astype(f).reshape(4, 128, CHUNK)
    sel = np.zeros((2, 128), f)
    sel[0, :64] = 1.0
    sel[1, 64:] = 1.0

    shared = dict(
        wq=wtile(Wq, ET), wk=wtile(Wk, ET), wv=wtile(Wv2, ET), wo=wtile(W_o, ET),
        wfc=wtile(W_fc, ET), wpr=wtile(W_proj, FT),
        bq=ptile(bqv, ET), bk=ptile(bkv, ET), bo=ptile(bo2, ET),
        bfc=ptile(b_fc, FT), bpr=ptile(b_proj, ET),
        g1=ptile(ln1_g, ET), b1=ptile(ln1_b, ET),
        g2=ptile(ln2_g, ET), b2=ptile(ln2_b, ET),
        causal=causal, sel=sel,
    )
    in_maps = []
    for core in range(NC):
        b, c = divmod(core, 4)
        P = CHUNK * (c + 1)
        xpad = np.zeros((PT, N_EMBD), f)
        xpad[PT - P:] = x[b, :P]
        m = dict(shared)
        m["xT"] = np.ascontiguousarray(xpad.T).reshape(ET, 128, PT)
        m["npad"] = np.full((1, 1), PT - P, f)
        in_maps.append(m)
    return in_maps


_CACHE = {}


def _get_built():
    if "nc" not in _CACHE:
        _CACHE["nc"] = build(stage=3, dbg=False, reps=1)[0]
    return _CACHE["nc"]


def kernel(**inputs):
    from concourse.bass_utils import run_bass_kernel_spmd

    nc = _get_built()
    in_maps = _prep_inputs(**inputs)
    res = run_bass_kernel_spmd(nc, in_maps, list(range(NC)))
    out = np.zeros((B, T, N_EMBD), np.float32)
    for core in range(NC):
        b, c = divmod(core, 4)
        yt = res.results[core]["yT"].reshape(N_EMBD, CHUNK)
        out[b, CHUNK * c:CHUNK * (c + 1), :] = yt.T
    return out


# revision 15
# speedup vs baseline: 5.1409x; 5.1409x over previous
"""Trainium2 Bass kernel for a pre-LN transformer decoder block.

Reference computation (per batch b):
    h  = LN1(x);  qkv = h@W_qkv + b;  causal attention (16 heads, d=64)
    x  = x + attn@W_o + b_o
    h  = LN2(x);  x = x + gelu(h@W_fc + b_fc)@W_proj + b_proj

Sharding: 8 cores = 2 batches x 4 query chunks of 512 tokens.  Each core
receives its batch's context front-padded with zeros to 2048 tokens
(queries always sit at local tokens 1536:2048), computes K/V for the whole
local context on-device, and emits the final 512 output rows.  No
cross-core communication.

Layout: everything feature-major (features on SBUF partitions), transposed
on the host.  Matmuls run as float32r (fp32 storage rounded to FP22 on
write, fp32 PSUM accumulate) — full PE rate at N>=256.  Scores are
computed transposed S^T[k, q] so softmax normalization folds into
per-partition ops + one tiny broadcast matmul.  Padded keys yield
exactly-zero scores; their exp(0)=1 contribution is removed by
subtracting the per-core pad count from the softmax denominator.
"""

import sys

if "/opt/trn_rl_repo" not in sys.path:
    sys.path.insert(0, "/opt/trn_rl_repo")

import numpy as np

N_EMBD = 1024
N_HEAD = 16
HEAD_DIM = 64
B, T = 2, 2048
NC = 8  # cores
CHUNK = 512  # query tokens per core
PT = 2048  # padded local context
ET = N_EMBD // 128  # 8 embedding tiles
FT = 4 * N_EMBD // 128  # 32 mlp tiles
KT = PT // 128  # 16 key tiles
TC = PT // 512  # 4 token chunks of 512
VW = 65 * N_HEAD  # 1040: v with interleaved ones columns
EPS = 1e-5


def build(stage=3, dbg=False, reps=1):
    import concourse.mybir as mybir
    import concourse.tile as tile
    from concourse import bacc

    f32 = mybir.dt.float32
    f32r = mybir.dt.float32r

    nc = bacc.Bacc("TRN2", target_bir_lowering=False, debug=False, num_devices=NC)

    g = {}
    g["xT"] = nc.declare_dram_parameter("xT", [ET, TC, 128, 512], f32, isOutput=False)
    g["wq"] = nc.declare_dram_parameter("wq", [4, 128, ET, 256], f32, isOutput=False)
    g["wk"] = nc.declare_dram_parameter("wk", [4, 128, ET, 256], f32, isOutput=False)
    g["wv"] = nc.declare_dram_parameter("wv", [5, 128, ET, 256], f32, isOutput=False)
    g["wo"] = nc.declare_dram_parameter("wo", [4, 128, ET, 256], f32, isOutput=False)
    g["wfc"] = nc.declare_dram_parameter("wfc", [16, 128, ET, 256], f32, isOutput=False)
    g["wpr"] = nc.declare_dram_parameter("wpr", [4, 4, 128, ET, 256], f32, isOutput=False)
    for nm, n in (("bq", ET), ("bk", ET), ("bo", ET), ("bfc", FT), ("bpr", ET),
                  ("g1", ET), ("b1", ET), ("g2", ET), ("b2", ET)):
        g[nm] = nc.declare_dram_parameter(nm, [128, n], f32, isOutput=False)
    g["causal"] = nc.declare_dram_parameter("causal", [4, 128, CHUNK], f32, isOutput=False)
    g["sel"] = nc.declare_dram_parameter("sel", [2, 128], f32, isOutput=False)
    g["npad"] = nc.declare_dram_parameter("npad", [1, 1], f32, isOutput=False)

    g["yT"] = nc.declare_dram_parameter("yT", [ET, 128, CHUNK], f32, isOutput=True)
    dbg_out = {}
    if dbg:
        dbg_out["lnxT"] = nc.declare_dram_parameter("d_lnxT", [ET, 128, PT], f32, isOutput=True)
        dbg_out["kT"] = nc.declare_dram_parameter("d_kT", [ET, 128, PT], f32, isOutput=True)
        dbg_out["qT"] = nc.declare_dram_parameter("d_qT", [ET, 128, CHUNK], f32, isOutput=True)
        dbg_out["v"] = nc.declare_dram_parameter("d_v", [KT, 128, VW], f32, isOutput=True)
        if stage >= 2:
            dbg_out["attnT"] = nc.declare_dram_parameter("d_attnT", [ET, 128, CHUNK], f32, isOutput=True)
            dbg_out["x2T"] = nc.declare_dram_parameter("d_x2T", [ET, 128, CHUNK], f32, isOutput=True)

    g["kts"] = nc.dram_tensor("kts", [ET, TC, 128, 512], f32r)  # K^T spill
    g["attns"] = nc.dram_tensor("attns", [ET, 128, CHUNK], f32r)  # attn^T spill

    with tile.TileContext(nc) as tc:
        with (
            tc.tile_pool(name="const", bufs=1) as cpool,
            tc.tile_pool(name="resident", bufs=1) as rpool,
        ):
            ones_tmp = cpool.tile([128, 16], f32, tag="ones_tmp", name="ones_tmp")
            nc.vector.memset(ones_tmp[:, :], 1.0)
            g["ones16"] = ones_tmp
            g["ones_p"] = cpool.tile([128, 1], f32r, tag="ones_p", name="ones_p")
            g["ones_f"] = cpool.tile([1, 128], f32r, tag="ones_f", name="ones_f")
            nc.vector.tensor_copy(g["ones_p"][:, :], ones_tmp[:, 0:1])
            nc.vector.tensor_copy(g["ones_f"][:, :], ones_tmp[0:1, 0:1].to_broadcast([1, 128]))
            g["eps_sb"] = cpool.tile([128, 1], f32, tag="eps_sb", name="eps_sb")
            nc.vector.memset(g["eps_sb"][:, :], EPS)
            for nm, shp, dt_ in (("sel_sb", [2, 128], f32r), ("np_sb", [1, 1], f32),
                                 ("cau_sb", [128, 4, CHUNK], f32),
                                 ("bq_sb", [128, ET], f32), ("bk_sb", [128, ET], f32),
                                 ("bo_sb", [128, ET], f32), ("bfc_sb", [128, FT], f32),
                                 ("bpr_sb", [128, ET], f32), ("g1_sb", [128, ET], f32),
                                 ("b1_sb", [128, ET], f32), ("g2_sb", [128, ET], f32),
                                 ("b2_sb", [128, ET], f32)):
                g[nm] = cpool.tile(shp, dt_, tag=nm, name=nm)
            f32r_tiles = {"sel_sb"}
            for s_, t_ in (("sel_sb", "sel"), ("np_sb", "npad"), ("cau_sb", "causal"),
                           ("bq_sb", "bq"), ("bk_sb", "bk"), ("bo_sb", "bo"),
                           ("bfc_sb", "bfc"), ("bpr_sb", "bpr"), ("g1_sb", "g1"),
                           ("b1_sb", "b1"), ("g2_sb", "g2"), ("b2_sb", "b2")):
                src = g[t_]
                dst = g[s_]
                src_ap = src[tuple(slice(None) for _ in src.shape)]
                if s_ == "cau_sb":
                    src_ap = src_ap.transpose([1, 0, 2])
                if s_ in f32r_tiles:
                    src_ap = src_ap.bitcast(f32r)
                nc.sync.dma_start(out=dst[tuple(slice(None) for _ in dst.shape)],
                                  in_=src_ap)

            with nc.allow_low_precision(reason="fp32r (fp22) matmul operands by design"):
                for _rep in range(reps):
                    _body(nc, tc, g, stage, dbg, dbg_out)

    nc.compile()
    return nc, dbg_out


def _body(nc, tc, g, stage, dbg, dbg_out):
    import concourse.mybir as mybir

    f32 = mybir.dt.float32
    f32r = mybir.dt.float32r
    AF = mybir.ActivationFunctionType
    OP = mybir.AluOpType

    def b_(ap):  # read an f32r AP on DVE/ACT
        return ap.bitcast(f32)

    from contextlib import ExitStack

    xT, kts, yT = g["xT"], g["kts"], g["yT"]
    ones_p, ones_f = g["ones_p"], g["ones_f"]
    qr_stack = ExitStack()
    qrpool = qr_stack.enter_context(tc.tile_pool(name="qkv_res", bufs=1))
    v_t = [qrpool.tile([128, VW], f32r, tag=f"v{i}", name=f"v{i}") for i in range(KT)]
    qT_t = [qrpool.tile([128, CHUNK], f32r, tag=f"qT{i}", name=f"qT{i}") for i in range(ET)]

    def kgetw(w, panel):
        return w[panel, :, :, :].bitcast(f32r)

    def layernorm_feature_major(pool, pspool, src_tiles_or_dram, width, out_tiles,
                                g_sb, b_sb, pfx, dram_mode):
        """LN over features (partitions); writes out_tiles[et][:, sl] (f32r)."""
        n_ch = width // 512
        for tci in range(n_ch):
            sl = slice(512 * tci, 512 * (tci + 1))

            def get_src(et):
                if dram_mode:
                    xt = pool.tile([128, 512], f32r, tag=pfx + "xt", name=pfx + "xt")
                    nc.sync.dma_start(out=xt[:, :], in_=src_tiles_or_dram[et, tci].bitcast(f32r))
                    return xt[:, :]
                return src_tiles_or_dram[et][:, sl]

            ps_s = pspool.tile([1, 512], f32, tag=pfx + "s", name=pfx + "s")
            ps_q = pspool.tile([1, 512], f32, tag=pfx + "q", name=pfx + "q")
            for et in range(ET):
                xt = get_src(et)
                sq = pool.tile([128, 512], f32r, tag=pfx + "sq", name=pfx + "sq")
                nc.vector.tensor_mul(sq[:, :], b_(xt), b_(xt))
                nc.tensor.matmul(ps_s[:, :], ones_p[:, :], xt,
                                 start=(et == 0), stop=(et == ET - 1))
                nc.tensor.matmul(ps_q[:, :], ones_p[:, :], sq[:, :],
                                 start=(et == 0), stop=(et == ET - 1))
            mu = pool.tile([1, 512], f32r, tag=pfx + "mu", name=pfx + "mu")
            nc.scalar.activation(mu[:, :], ps_s[:, :], AF.Copy, scale=1.0 / N_EMBD)
            musq = pool.tile([1, 512], f32, tag=pfx + "musq", name=pfx + "musq")
            nc.vector.tensor_mul(musq[:, :], b_(mu[:, :]), b_(mu[:, :]))
            var = pool.tile([1, 512], f32, tag=pfx + "var", name=pfx + "var")
            nc.scalar.activation(var[:, :], ps_q[:, :], AF.Copy, scale=1.0 / N_EMBD)
            nc.vector.tensor_sub(var[:, :], var[:, :], musq[:, :])
            sd = pool.tile([1, 512], f32, tag=pfx + "sd", name=pfx + "sd")
            nc.scalar.activation(sd[:, :], var[:, :], AF.Sqrt, bias=g["eps_sb"][0:1, 0:1])
            rstd = pool.tile([1, 512], f32r, tag=pfx + "rstd", name=pfx + "rstd")
            nc.vector.reciprocal(rstd[:, :], sd[:, :])
            ps_mb = pspool.tile([128, 512], f32, tag=pfx + "mb", name=pfx + "mb")
            ps_rb = pspool.tile([128, 512], f32, tag=pfx + "rb", name=pfx + "rb")
            nc.tensor.matmul(ps_mb[:, :], ones_f[:, :], mu[:, :], start=True, stop=True)
            nc.tensor.matmul(ps_rb[:, :], ones_f[:, :], rstd[:, :], start=True, stop=True)
            mu_b = pool.tile([128, 512], f32, tag=pfx + "mu_b", name=pfx + "mu_b")
            rs_b = pool.tile([128, 512], f32, tag=pfx + "rs_b", name=pfx + "rs_b")
            nc.vector.tensor_copy(mu_b[:, :], ps_mb[:, :])
            nc.vector.tensor_copy(rs_b[:, :], ps_rb[:, :])
            for et in range(ET):
                xt = get_src(et)
                t1 = pool.tile([128, 512], f32, tag=pfx + "t1", name=pfx + "t1")
                nc.vector.tensor_sub(t1[:, :], b_(xt), mu_b[:, :])
                nc.vector.tensor_mul(t1[:, :], t1[:, :], rs_b[:, :])
                nc.vector.tensor_scalar(out_tiles[et][:, sl], t1[:, :],
                                        g_sb[:, et:et + 1], b_sb[:, et:et + 1],
                                        OP.mult, OP.add)

    # ---------------- Phase A: LN1 + QKV ----------------
    with tc.tile_pool(name="lnx_pool", bufs=1) as lpool:
        lnxT = [lpool.tile([128, PT], f32r, tag=f"lnxT{i}", name=f"lnxT{i}") for i in range(ET)]
        with (
            tc.tile_pool(name="pa_sb", bufs=2) as pa,
            tc.tile_pool(name="pa_ps", bufs=2, space="PSUM") as pps,
        ):
            layernorm_feature_major(pa, pps, xT, PT, lnxT,
                                    g["g1_sb"], g["b1_sb"], "a", dram_mode=True)

        with (
            tc.tile_pool(name="qkv_w", bufs=2) as wpool,
            tc.tile_pool(name="qkv_sb", bufs=4) as spool,
            tc.tile_pool(name="qkv_ps", bufs=4, space="PSUM") as qps,
        ):
            for panel in range(4):
                wk_p = wpool.tile([128, ET, 256], f32r, tag="wpan", name="wk_p")
                nc.sync.dma_start(out=wk_p[:, :, :], in_=kgetw(g["wk"], panel))
                for fto in range(2):
                    ft = 2 * panel + fto
                    for tci in range(TC):
                        ps = qps.tile([128, 512], f32, tag="mmps", name="kq_ps")
                        for et in range(ET):
                            nc.tensor.matmul(ps[:, :], wk_p[:, et, 128 * fto:128 * (fto + 1)],
                                             lnxT[et][:, 512 * tci:512 * (tci + 1)],
                                             start=(et == 0), stop=(et == ET - 1))
                        kt_sb = spool.tile([128, 512], f32r, tag="kt_sb", name="kt_sb")
                        nc.vector.tensor_scalar(kt_sb[:, :], ps[:, :],
                                                g["bk_sb"][:, ft:ft + 1], None, OP.add)
                        nc.sync.dma_start(out=kts[ft, tci], in_=kt_sb[:, :])
                        if dbg:
                            nc.sync.dma_start(out=dbg_out["kT"][ft, :, 512 * tci:512 * (tci + 1)],
                                              in_=b_(kt_sb[:, :]))  # dbg only
                wq_p = wpool.tile([128, ET, 256], f32r, tag="wpan", name="wq_p")
                nc.sync.dma_start(out=wq_p[:, :, :], in_=kgetw(g["wq"], panel))
                for fto in range(2):
                    ft = 2 * panel + fto
                    ps = qps.tile([128, 512], f32, tag="mmps", name="q_ps")
                    for et in range(ET):
                        nc.tensor.matmul(ps[:, :], wq_p[:, et, 128 * fto:128 * (fto + 1)],
                                         lnxT[et][:, 1536:2048],
                                         start=(et == 0), stop=(et == ET - 1))
                    nc.vector.tensor_scalar(qT_t[ft][:, :], ps[:, :],
                                            g["bq_sb"][:, ft:ft + 1], None, OP.add)
                    if dbg:
                        nc.sync.dma_start(out=dbg_out["qT"][ft, :, :], in_=b_(qT_t[ft][:, :]))
            for vp, (c0, cw) in enumerate(((0, 256), (256, 256), (512, 256), (768, 256), (1024, 16))):
                wv_p = wpool.tile([128, ET, 256], f32r, tag="wpan", name="wv_p")
                nc.sync.dma_start(out=wv_p[:, :, :], in_=kgetw(g["wv"], vp))
                for tt in range(KT):
                    ps = qps.tile([128, 256], f32, tag="vps", name="v_ps")
                    for et in range(ET):
                        nc.tensor.matmul(ps[:, :cw], lnxT[et][:, 128 * tt:128 * (tt + 1)],
                                         wv_p[:, et, :cw],
                                         start=(et == 0), stop=(et == ET - 1))
                    nc.vector.tensor_copy(v_t[tt][:, c0:c0 + cw], ps[:, :cw])
            for tt in range(KT):  # ones columns feeding the softmax denominator
                nc.vector.tensor_copy(v_t[tt][:, 64:VW:65], g["ones16"][:, :])
                if dbg:
                    nc.sync.dma_start(out=dbg_out["v"][tt, :, :], in_=b_(v_t[tt][:, :]))
            if dbg:
                for et in range(ET):
                    nc.sync.dma_start(out=dbg_out["lnxT"][et, :, :], in_=b_(lnxT[et][:, :]))

    if stage < 2:
        for et in range(ET):
            nc.sync.dma_start(out=yT[et, :, :], in_=b_(qT_t[et][:, :]))
        qr_stack.close()
        return

    # ---------------- Phase B: attention ----------------
    attns = g["attns"]
    if True:
        with (
            tc.tile_pool(name="at_sb", bufs=2) as apool,
            tc.tile_pool(name="at_exp", bufs=3) as epool,
            tc.tile_pool(name="at_ps", bufs=2, space="PSUM") as sps,
            tc.tile_pool(name="at_av", bufs=2, space="PSUM") as avps,
            tc.tile_pool(name="at_bc", bufs=1, space="PSUM") as bcps,
        ):
            for hp in range(N_HEAD // 2):
                kp = apool.tile([128, PT], f32r, tag="kp", name="kp")
                for tci in range(TC):
                    nc.sync.dma_start(out=kp[:, 512 * tci:512 * (tci + 1)], in_=kts[hp, tci])
                ps_a = [None, None]
                recip_t = [None, None]
                for h2 in range(2):
                    po = 64 * h2
                    h = 2 * hp + h2
                    ps_a[h2] = avps.tile([65, 512], f32, tag="av", name="av_ps")
                    for kt2 in range(KT // 2):
                        ps_s = sps.tile([128, 1024], f32, tag="scores", name="s_ps")
                        ex = epool.tile([128, 1024], f32r, tag="expS", name="expS")
                        for j in range(2):
                            kt = 2 * kt2 + j
                            nc.tensor.matmul(ps_s[:, 512 * j:512 * (j + 1)],
                                             kp[po:po + 64, 128 * kt:128 * (kt + 1)],
                                             qT_t[hp][po:po + 64, :],
                                             start=True, stop=True)
                            if kt >= 12:
                                nc.vector.tensor_add(ps_s[:, 512 * j:512 * (j + 1)],
                                                     ps_s[:, 512 * j:512 * (j + 1)],
                                                     g["cau_sb"][:, kt - 12, :])
                        nc.scalar.activation(ex[:, :], ps_s[:, :], AF.Exp)
                        for j in range(2):
                            kt = 2 * kt2 + j
                            nc.tensor.matmul(ps_a[h2][:, :],
                                             v_t[kt][:, 65 * h:65 * h + 65],
                                             ex[:, 512 * j:512 * (j + 1)],
                                             start=(kt == 0), stop=(kt == KT - 1),
                                             skip_group_check=True)
                    den = apool.tile([1, 512], f32, tag="den", name="den")
                    nc.vector.tensor_scalar(den[:, :], ps_a[h2][64:65, :],
                                            g["np_sb"][0:1, 0:1], None, OP.subtract)
                    recip_t[h2] = apool.tile([1, 512], f32r, tag=f"recip{h2}", name=f"recip{h2}")
                    nc.vector.reciprocal(recip_t[h2][:, :], den[:, :])
                rb_sb = apool.tile([128, 512], f32, tag="rb_sb", name="rb_sb")
                for h2 in range(2):
                    ps_b = bcps.tile([64, 512], f32, tag="bc", name="bc_ps")
                    nc.tensor.matmul(ps_b[:, :], ones_f[0:1, 0:64],
                                     recip_t[h2][:, :], start=True, stop=True)
                    nc.vector.tensor_copy(rb_sb[64 * h2:64 * h2 + 64, :], ps_b[:, :])
                at_sb = apool.tile([128, CHUNK], f32r, tag="at_sb_t", name="at_sb_t")
                for h2 in range(2):
                    po = 64 * h2
                    nc.vector.tensor_mul(at_sb[po:po + 64, :], ps_a[h2][0:64, :],
                                         rb_sb[po:po + 64, :])
                nc.sync.dma_start(out=attns[hp, :, :], in_=at_sb[:, :])
                if dbg:
                    nc.sync.dma_start(out=dbg_out["attnT"][hp, :, :], in_=b_(at_sb[:, :]))
        qr_stack.close()  # v/q dead from here; free 82KB for phase C

        # ---------------- Phase C: W_o + residual + LN2 + MLP ----------------
        with (
            tc.tile_pool(name="pc_sb", bufs=2) as pc,
            tc.tile_pool(name="pc_res", bufs=1) as pcr,
            tc.tile_pool(name="pc_w", bufs=2) as pw,
            tc.tile_pool(name="pc_ps", bufs=4, space="PSUM") as cps,
            tc.tile_pool(name="pc_ps2", bufs=1, space="PSUM") as cps2,
        ):
            x2T = [pcr.tile([128, CHUNK], f32r, tag=f"x2T{i}", name=f"x2T{i}") for i in range(ET)]
            ln2T = [pcr.tile([128, CHUNK], f32r, tag=f"ln2T{i}", name=f"ln2T{i}") for i in range(ET)]
            hT = [pcr.tile([128, CHUNK], f32r, tag=f"hT{i}", name=f"hT{i}") for i in range(FT)]
            attnT = [pcr.tile([128, CHUNK], f32r, tag=f"atT{i}", name=f"atT{i}") for i in range(ET)]
            for et in range(ET):
                nc.sync.dma_start(out=attnT[et][:, :], in_=attns[et, :, :])
            for panel in range(4):
                wo_p = pw.tile([128, ET, 256], f32r, tag="cwpan", name="wo_p")
                nc.sync.dma_start(out=wo_p[:, :, :], in_=kgetw(g["wo"], panel))
                for fto in range(2):
                    ot = 2 * panel + fto
                    ps = cps.tile([128, 512], f32, tag="cmmps", name="wo_ps")
                    for et in range(ET):
                        nc.tensor.matmul(ps[:, :], wo_p[:, et, 128 * fto:128 * (fto + 1)],
                                         attnT[et][:, :],
                                         start=(et == 0), stop=(et == ET - 1))
                    xq = pc.tile([128, 512], f32, tag="xq", name="xq")
                    nc.sync.dma_start(out=xq[:, :], in_=xT[ot, TC - 1])
                    nc.vector.scalar_tensor_tensor(x2T[ot][:, :], ps[:, :],
                                                   g["bo_sb"][:, ot:ot + 1], xq[:, :],
                                                   OP.add, OP.add)
            if dbg:
                for et in range(ET):
                    nc.sync.dma_start(out=dbg_out["x2T"][et, :, :], in_=b_(x2T[et][:, :]))

            layernorm_feature_major(pc, cps2, x2T, CHUNK, ln2T,
                                    g["g2_sb"], g["b2_sb"], "c", dram_mode=False)

            for panel in range(16):
                wf_p = pw.tile([128, ET, 256], f32r, tag="cwpan", name="wf_p")
                nc.sync.dma_start(out=wf_p[:, :, :], in_=kgetw(g["wfc"], panel))
                for fto in range(2):
                    ft = 2 * panel + fto
                    ps = cps.tile([128, 512], f32, tag="cmmps", name="fc_ps")
                    for et in range(ET):
                        nc.tensor.matmul(ps[:, :], wf_p[:, et, 128 * fto:128 * (fto + 1)],
                                         ln2T[et][:, :],
                                         start=(et == 0), stop=(et == ET - 1))
                    nc.scalar.activation(hT[ft][:, :], ps[:, :], AF.Gelu,
                                         bias=g["bfc_sb"][:, ft:ft + 1])

            for panel in range(4):  # proj output features, 256 (= 2 e-tiles) each
                pso = [cps.tile([128, 512], f32, tag="cmmps", name=f"pr_ps{i}") for i in range(2)]
                for ftg in range(4):
                    wp_p = pw.tile([128, ET, 256], f32r, tag="cwpan", name="wp_p")
                    nc.sync.dma_start(out=wp_p[:, :, :],
                                      in_=g["wpr"][panel, ftg].bitcast(f32r))
                    for fl in range(ET):
                        ft = 8 * ftg + fl
                        for fto in range(2):
                            nc.tensor.matmul(pso[fto][:, :],
                                             wp_p[:, fl, 128 * fto:128 * (fto + 1)],
                                             hT[ft][:, :],
                                             start=(ft == 0), stop=(ft == FT - 1),
                                             skip_group_check=True)
                for fto in range(2):
                    et_o = 2 * panel + fto  # output e-tile index
                    out_sb = pc.tile([128, 512], f32, tag="out_sb", name="out_sb")
                    nc.vector.scalar_tensor_tensor(out_sb[:, :], pso[fto][:, :],
                                                   g["bpr_sb"][:, et_o:et_o + 1],
                                                   b_(x2T[et_o][:, :]), OP.add, OP.add)
                    nc.sync.dma_start(out=yT[et_o, :, :], in_=out_sb[:, :])


def _prep_inputs(x, ln1_g, ln1_b, ln2_g, ln2_b, W_qkv, b_qkv, W_o, b_o, W_fc, b_fc, W_proj, b_proj):
    """Host-side shard prep. Returns list of 8 in_maps."""
    f = np.float32
    x = np.asarray(x, f)
    W_qkv = np.asarray(W_qkv, f)
    b_qkv = np.asarray(b_qkv, f)
    scale = f(1.0) / f(np.sqrt(HEAD_DIM))
    Wq = W_qkv[:, :N_EMBD] * scale
    Wk = W_qkv[:, N_EMBD:2 * N_EMBD]
    Wv = W_qkv[:, 2 * N_EMBD:]
    bqv = b_qkv[:N_EMBD] * scale
    bkv = b_qkv[N_EMBD:2 * N_EMBD]
    bvv = b_qkv[2 * N_EMBD:]
    Wv2 = np.zeros((N_EMBD, VW), f)
    for h in range(N_HEAD):
        Wv2[:, 65 * h:65 * h + 64] = Wv[:, 64 * h:64 * h + 64]
    bo2 = np.asarray(b_o, f) + bvv @ np.asarray(W_o, f)

    def ptile(vec, n):
        return np.ascontiguousarray(np.asarray(vec, f).reshape(n, 128).T)

    def wpanels(w, n_et, width=256):
        w3 = np.asarray(w, f).reshape(n_et, 128, -1)
        cols = w3.shape[2]
        pans = [w3[:, :, i:i + width].transpose(1, 0, 2) for i in range(0, cols, width)]
        return np.ascontiguousarray(np.stack(pans))

    causal = np.where(np.arange(CHUNK)[:, None] > np.arange(CHUNK)[None, :],
                      f(-1e9), f(0.0)).astype(f).reshape(4, 128, CHUNK)
    sel = np.zeros((2, 128), f)
    sel[0, :64] = 1.0
    sel[1, 64:] = 1.0

    Wv2p = np.zeros((N_EMBD, 1280), f)
    Wv2p[:, :VW] = Wv2
    w3p = np.asarray(W_proj, f).reshape(FT, 128, N_EMBD)
    wpr4 = np.ascontiguousarray(np.stack([
        np.stack([w3p[8 * g_:8 * g_ + 8, :, 256 * p_:256 * (p_ + 1)].transpose(1, 0, 2)
                  for g_ in range(4)]) for p_ in range(4)]))
    shared = dict(
        wq=wpanels(Wq, ET), wk=wpanels(Wk, ET), wv=wpanels(Wv2p, ET), wo=wpanels(W_o, ET),
        wfc=wpanels(W_fc, ET), wpr=wpr4,
        bq=ptile(bqv, ET), bk=ptile(bkv, ET), bo=ptile(bo2, ET),
        bfc=ptile(b_fc, FT), bpr=ptile(b_proj, ET),
        g1=ptile(ln1_g, ET), b1=ptile(ln1_b, ET),
        g2=ptile(ln2_g, ET), b2=ptile(ln2_b, ET),
        causal=causal, sel=sel,
    )
    in_maps = []
    for core in range(NC):
        b, c = divmod(core, 4)
        P = CHUNK * (c + 1)
        xpad = np.zeros((PT, N_EMBD), f)
        xpad[PT - P:] = x[b, :P]
        m = dict(shared)
        m["xT"] = np.ascontiguousarray(
            xpad.T.reshape(ET, 128, TC, 512).transpose(0, 2, 1, 3))
        m["npad"] = np.full((1, 1), PT - P, f)
        in_maps.append(m)
    return in_maps


_CACHE = {}


def _get_built():
    if "nc" not in _CACHE:
        _CACHE["nc"] = build(stage=3, dbg=False, reps=1)[0]
    return _CACHE["nc"]


def kernel(**inputs):
    from concourse.bass_utils import run_bass_kernel_spmd

    nc = _get_built()
    in_maps = _prep_inputs(**inputs)
    res = run_bass_kernel_spmd(nc, in_maps, list(range(NC)))
    out = np.zeros((B, T, N_EMBD), np.float32)
    for core in range(NC):
        b, c = divmod(core, 4)
        yt = res.results[core]["yT"].reshape(N_EMBD, CHUNK)
        out[b, CHUNK * c:CHUNK * (c + 1), :] = yt.T
    return out
